# revision 26
# baseline (speedup 1.0000x reference)
"""Trainium2 Bass kernel for AGCNODEFunc (gnn_message_passing).

f = tanh(xe + 0.5*a*xa + x@W + x0*sig(beta) - 3x) where
  adj = softmax(relu(emb@emb.T), axis=1); xa = cw*(adj@x)+cb
  S[n,k] = sigmoid(e1[n]e2[k] + bs[n,k]); M = vs@S; Emat = softmax(M, -1); xe = Emat@x

Sharding: 8 cores = 4 batches x 2 row-halves (fully data-parallel).

v4: everything computed TRANSPOSED (no PE transposes); the N^3 matmul in
fp8 DoubleRow. Relative to v3:
  - phase A (adj@x: z = emb@emb^T, exp, u = [x|1]^T @ exp(relu(z))) is
    INTERLEAVED into the main MT sweep at accumulation-group boundaries,
    so the PE never idles and HAM stays at K=8/8 (v3 ran the whole phase
    at half clock: 192us of K=4/8).
  - MT PSUM pairs into one (128,1024) tile spanning 2 banks -> ONE
    1024-wide exp per (kb,pp) group ((N+352)/1.2 scalar cost amortized).
  - S' arg planes split: i=0 on DVE, i=1 on GpSimd.
  - bs/vs DMAs consolidated to one (128,2,*) DMA per pair-tile; x tiles
    for the xe/u stationaries land in ONE 512KB DMA (host pre-casts bf16).
  - xa fold + epilogue use broadcast-FIRST reciprocals ((128,2048) DVE
    reciprocal is 1us; v3's single-lane (1,2048) was 15.7us) and the
    softmax denominator row is broadcast by a K=1 ones matmul on the PE
    (v3 did two DRAM round-trips).
Softmax over k needs NO max pass: logits 0.5*M lie in [-140, 140], so
exp(0.5*MT - 64) neither overflows f32 nor flushes whole rows.
Output (F, MH) transposed; host transposes back.
"""

import numpy as np
import ml_dtypes

import concourse.bass as bass
import concourse.bacc as bacc
import concourse.mybir as mybir
from concourse import tile
from concourse.bass_utils import run_bass_kernel_spmd

B, N, F, E = 4, 4096, 64, 16
P = 128
MH = N // 2            # 2048 m-rows per core
KS = 512               # k-strip width
NSTR = N // KS         # 8 strips
NS2 = N // 256         # 16 pair-subtiles over n (contraction)
XT = N // P            # 32 x tiles
CSH = 64.0             # softmax constant shift (see module docstring)
f32 = mybir.dt.float32
bf16 = mybir.dt.bfloat16
fp8 = mybir.dt.float8e4
AF = mybir.ActivationFunctionType
ALU = mybir.AluOpType
DR = mybir.MatmulPerfMode.DoubleRow

_CACHE = {}
DEBUG = False


def build_nc():
    nc = bacc.Bacc()
    d_xb = nc.dram_tensor("xb", (N, F), bf16, kind="ExternalInput")
    d_e2b = nc.dram_tensor("e2b", (P, N), bf16, kind="ExternalInput")
    d_e1c = nc.dram_tensor("e1c", (P, XT), f32, kind="ExternalInput")
    d_Wsb = nc.dram_tensor("Wsb", (F, F), f32, kind="ExternalInput")
    d_xhT = nc.dram_tensor("xhT", (F, MH), f32, kind="ExternalInput")
    d_x0T = nc.dram_tensor("x0T", (F, MH), f32, kind="ExternalInput")
    d_alr = nc.dram_tensor("alr", (1, MH), bf16, kind="ExternalInput")
    d_ber = nc.dram_tensor("ber", (1, MH), bf16, kind="ExternalInput")
    d_cv = nc.dram_tensor("conv2", (1, 2), f32, kind="ExternalInput")
    d_vs8 = nc.dram_tensor("vs8", (N, MH), fp8, kind="ExternalInput")
    d_bs16 = nc.dram_tensor("bs16", (N, N), bf16, kind="ExternalInput")
    d_embT = nc.dram_tensor("embT", (E, N), bf16, kind="ExternalInput")
    d_embhT = nc.dram_tensor("emb_hT", (E, MH), bf16, kind="ExternalInput")
    d_out = nc.dram_tensor("out", (F, MH), f32, kind="ExternalOutput")
    if DEBUG:
        d_dbg_e2b = nc.dram_tensor("dbg_e2b", (P, N), bf16,
                                   kind="ExternalOutput")
        d_dbg_e12 = nc.dram_tensor("dbg_e12", (P, 2 * XT), f32,
                                   kind="ExternalOutput")
        d_dbg_u = nc.dram_tensor("dbg_u", (F + 1, MH), f32,
                                 kind="ExternalOutput")
        d_dbg_xeT = nc.dram_tensor("dbg_xeT", (F + 1, MH), f32,
                                   kind="ExternalOutput")
        d_dbg_rest = nc.dram_tensor("dbg_rest", (F, MH), f32,
                                    kind="ExternalOutput")
        d_dbg_xeb = nc.dram_tensor("dbg_xeb", (P, XT * (F + 1)), bf16,
                                   kind="ExternalOutput")

    with tile.TileContext(nc) as tc:
        with (
            tc.tile_pool(name="persist", bufs=1) as persist,
            tc.tile_pool(name="vspool", bufs=1) as vspool,
            tc.tile_pool(name="spool", bufs=1) as spool,
            tc.tile_pool(name="bsq", bufs=2) as bsqp,
            tc.tile_pool(name="work", bufs=3) as workp,
            tc.tile_pool(name="exp", bufs=3) as expp,
            tc.tile_pool(name="ez", bufs=7) as ezp,
            tc.tile_pool(name="rows", bufs=4) as rowsp,
            tc.tile_pool(name="bcast", bufs=2) as bcp,
            tc.tile_pool(name="xrot", bufs=2) as xrot,
            # PSUM: mt2 2x2 banks + shared 3 + ups 1 = 8 banks
            tc.tile_pool(name="ps_mt2", bufs=2, space="PSUM") as ps_mt2,
            tc.tile_pool(name="ps_sh", bufs=3, space="PSUM") as ps_sh,
            tc.tile_pool(name="ps_ups", bufs=1, space="PSUM") as ps_ups,
        ):
            # ---------- persistent tiles ----------
            e2b = persist.tile([P, N], bf16)          # e2 bcast over partitions
            nshift = persist.tile([P, 1], f32)        # exp bias = -CSH
            nc.vector.memset(nshift[:], -CSH)
            ones1 = persist.tile([1, P], bf16)        # K=1 bcast stationary
            nc.vector.memset(ones1[:], 1.0)
            ones1f = persist.tile([F + 1, P], f32)    # row F used (part. 64)
            nc.vector.memset(ones1f[F:F + 1, :], 1.0)
            e1c = persist.tile([P, XT], f32)          # col j = e1 of n-block j
            cv = persist.tile([1, 2], f32)
            nc.sync.dma_start(cv[:], d_cv[:])
            cvb = persist.tile([P, 2], f32)
            nc.gpsimd.partition_broadcast(cvb[:], cv[:])
            # stationaries for xe/u matmuls: [x|1] bf16, (128, 32, 65)
            xeb = persist.tile([P, XT, F + 1], bf16)
            restT = persist.tile([F, MH], f32)
            xeT = persist.tile([F + 1, MH], f32)
            uT = persist.tile([F + 1, MH], f32)
            embT = persist.tile([E, N], bf16)
            embhT = persist.tile([E, MH], bf16)
            # vs^T fp8 pair tiles: vsT[j][p, i, m] = vs[m, j*256 + i*128 + p]
            vsT = [vspool.tile([P, 2, MH], fp8, tag=f"vsT{j}", name=f"vsT{j}")
                   for j in range(NS2)]
            # S' fp8 double-buffered strip tiles
            S8 = [[spool.tile([P, 2, KS], fp8, tag=f"S{par}_{j}",
                              name=f"S{par}_{j}") for j in range(NS2)]
                  for par in range(2)]

            # ---------- head DMAs (small first) ----------
            nc.scalar.dma_start(embT[:], d_embT[:])
            nc.scalar.dma_start(embhT[:], d_embhT[:])
            Wsb = persist.tile([F, F], f32)
            nc.sync.dma_start(Wsb[:], d_Wsb[:])
            nc.sync.dma_start(e1c[:], d_e1c[:])
            alr = rowsp.tile([1, MH], bf16, tag="row", name="alr")
            nc.sync.dma_start(alr[:], d_alr[:])
            ber = rowsp.tile([1, MH], bf16, tag="row", name="ber")
            nc.sync.dma_start(ber[:], d_ber[:])
            # all 32 x tiles in one DMA; ones column via strided memset
            nc.scalar.dma_start(xeb[:, :, :F],
                                d_xb[:].rearrange("(k p) f -> p k f", p=P))
            nc.vector.memset(xeb[:, :, F:F + 1], 1.0)
            nc.scalar.dma_start(e2b[:], d_e2b[:])

            # ---------- strip production: S' = tanh(0.5(e1 e2^T + bs)) ------
            def produce_j(s, j):
                par = s % 2
                k0 = s * KS
                if True:
                    bsq = bsqp.tile([P, 2, KS], bf16, tag="bsq", name="bsq")
                    nc.sync.dma_start(
                        bsq[:],
                        d_bs16[j * 256:(j + 1) * 256, k0:k0 + KS]
                        .rearrange("(i p) k -> p i k", p=P))
                    arg = workp.tile([P, 2, KS], bf16, tag="arg", name="arg")
                    for i, eng in ((0, nc.vector), (1, nc.vector)):
                        eng.scalar_tensor_tensor(
                            arg[:, i, :], e2b[:, k0:k0 + KS],
                            e1c[:, (2 * j + i):(2 * j + i) + 1],
                            bsq[:, i, :], op0=ALU.mult, op1=ALU.add)
                    nc.scalar.activation(S8[par][j][:], arg[:], AF.Tanh,
                                         scale=0.5)

            def produce(s):
                for j in range(NS2):
                    produce_j(s, j)

            produce(0)

            # vs8 ahead of restT inputs (needed by sweep group 0);
            # m-halves split so group 0 (m 0:1024) unblocks at half the bytes
            for mh in range(2):
                for j in range(NS2):
                    nc.gpsimd.dma_start(
                        vsT[j][:, :, mh * 1024:(mh + 1) * 1024],
                        d_vs8[j * 256:(j + 1) * 256,
                              mh * 1024:(mh + 1) * 1024]
                        .rearrange("(i p) m -> p i m", p=P))

            # ---------- restT = xw^T + x0^T*sig(beta) - 3x^T ----------
            sbr = rowsp.tile([1, MH], bf16, tag="row", name="sbr")
            nc.scalar.activation(sbr[:], ber[:], AF.Sigmoid)
            REST_G = {1: 0, 3: 1, 5: 2, 7: 3}

            def rest_q(q):
                sl = slice(q * KS, (q + 1) * KS)
                # broadcast sig(beta) chunk via K=1 ones matmul (gpsimd
                # partition_broadcast mishandles src free-offsets on HW)
                sbps = ps_sh.tile([P, KS], f32, tag="sh", name="sbps")
                nc.tensor.matmul(sbps[:], ones1[:], sbr[:, sl],
                                 start=True, stop=True)
                xhc = xrot.tile([F, KS], f32, tag="xc", name="xhc")
                nc.sync.dma_start(xhc[:], d_xhT[:, sl])
                x0c = xrot.tile([F, KS], f32, tag="x0c", name="x0c")
                nc.sync.dma_start(x0c[:], d_x0T[:, sl])
                xwps = ps_sh.tile([P, KS], f32, tag="sh", name="xwps")
                nc.tensor.matmul(xwps[:F, :], Wsb[:], xhc[:],
                                 start=True, stop=True)
                nc.vector.scalar_tensor_tensor(
                    restT[:, sl], xhc[:], -3.0, xwps[:F, :],
                    op0=ALU.mult, op1=ALU.add)
                t0 = workp.tile([F, KS], f32, tag="fin", name="t0")
                nc.vector.tensor_tensor(t0[:], x0c[:], sbps[:F, :],
                                        op=ALU.mult)
                nc.vector.tensor_tensor(restT[:, sl], restT[:, sl], t0[:],
                                        op=ALU.add)

            nc.vector.memset(xeT[:], 0.0)

            # ---------- phase A ops interleaved into the sweep ----------
            # z[i]: zps = embT_ns^T @ embh_mb ; ez = max(exp(zps),1)
            # u[i]: ups_mb += [x|1]_ns^T @ ez   (32 accumulating MMs per mb)
            NPA = 4 * XT                         # 128 z ops / 128 u ops
            PA_G0, PA_G1 = 8, 58                 # groups of strips 1..7
            z_sched = {}
            u_sched = {}
            for i in range(NPA):
                g = PA_G0 + (i * (PA_G1 - PA_G0)) // NPA
                z_sched.setdefault(g, []).append(i)
                u_sched.setdefault(g + 2, []).append(i)
            ez_buf = {}
            ups_cur = [None]

            def pa_zu(gi):
                for i in u_sched.get(gi, ()):
                    mb, ns = i // XT, i % XT
                    if ns == 0:
                        ups_cur[0] = ps_ups.tile([F + 1, KS], f32, tag="UPS",
                                                 name="upsT")
                    nc.tensor.matmul(ups_cur[0][:], xeb[:, ns, :],
                                     ez_buf.pop(i)[:],
                                     start=(ns == 0), stop=(ns == XT - 1))
                    if ns == XT - 1:
                        nc.vector.tensor_copy(
                            uT[:, mb * KS:(mb + 1) * KS], ups_cur[0][:])
                for i in z_sched.get(gi, ()):
                    mb, ns = i // XT, i % XT
                    zps = ps_sh.tile([P, KS], f32, tag="sh", name="zps")
                    nc.tensor.matmul(zps[:], embT[:, ns * P:(ns + 1) * P],
                                     embhT[:, mb * KS:(mb + 1) * KS],
                                     start=True, stop=True)
                    ez = ezp.tile([P, KS], bf16, tag="ez", name="ez")
                    nc.scalar.activation(ez[:], zps[:], AF.Exp)
                    nc.vector.tensor_scalar_max(ez[:], ez[:], 1.0)
                    ez_buf[i] = ez

            # fold xa chunk q: rest += (0.5*sa*cw/urow)*u[:F] + 0.5*sa*cb
            # (chunk q only needs mb=q's u columns; interleaved into strip 7)
            sar = rowsp.tile([1, MH], bf16, tag="row", name="sar")
            nc.scalar.activation(sar[:], alr[:], AF.Sigmoid)
            FOLD_G = {28: 0, 42: 1, 54: 2, 60: 3}

            def fold_q(q):
                sl = slice(q * KS, (q + 1) * KS)
                saps = ps_sh.tile([P, KS], f32, tag="sh", name="saps")
                nc.tensor.matmul(saps[:], ones1[:], sar[:, sl],
                                 start=True, stop=True)
                urps = ps_sh.tile([P, KS], f32, tag="sh", name="urps")
                nc.tensor.matmul(urps[:], ones1f[F:F + 1, :],
                                 uT[F:F + 1, sl], start=True, stop=True)
                s1b = bcp.tile([P, KS], f32, tag="bcf", name="s1b")
                nc.vector.reciprocal_approx_fast(s1b[:], urps[:])
                nc.vector.tensor_tensor(s1b[:], saps[:], s1b[:],
                                        op=ALU.mult)
                nc.vector.tensor_scalar(s1b[:], s1b[:], cvb[:, 0:1], 0.5,
                                        op0=ALU.mult, op1=ALU.mult)
                s0b = bcp.tile([P, KS], bf16, tag="bc", name="s0b")
                nc.vector.tensor_scalar(s0b[:], saps[:], cvb[:, 1:2], 0.5,
                                        op0=ALU.mult, op1=ALU.mult)
                t1 = workp.tile([F, KS], f32, tag="fin", name="t1")
                nc.vector.tensor_tensor(t1[:], uT[:F, sl], s1b[:F, :],
                                        op=ALU.mult)
                nc.vector.tensor_tensor(t1[:], t1[:], s0b[:F, :],
                                        op=ALU.add)
                nc.vector.tensor_tensor(restT[:, sl], restT[:, sl],
                                        t1[:], op=ALU.add)

            # ---------- main sweep: MT = S'^T vs^T (fp8 DoubleRow) ----------
            pend = []                # FIFO of (ksub, q4, Et2, h)

            def flush_one():
                ksub, q4, Et2, h = pend.pop(0)
                xeps = ps_sh.tile([F + 1, KS], f32, tag="sh", name="xeps")
                nc.tensor.matmul(xeps[:], xeb[:, ksub, :],
                                 Et2[:, h * KS:(h + 1) * KS],
                                 start=True, stop=True)
                nc.vector.tensor_tensor(
                    xeT[:, q4 * KS:(q4 + 1) * KS],
                    xeT[:, q4 * KS:(q4 + 1) * KS], xeps[:], op=ALU.add)

            for s in range(NSTR):
                Scur = S8[s % 2]
                for kb in range(4):
                    ksub = 4 * s + kb
                    for pp_ in range(2):
                        gi = s * 8 + kb * 2 + pp_
                        gl = kb * 2 + pp_
                        if s < NSTR - 1:
                            produce_j(s + 1, 2 * gl)
                            produce_j(s + 1, 2 * gl + 1)
                        pa_zu(gi)
                        if gi in REST_G:
                            rest_q(REST_G[gi])
                        if gi in FOLD_G:
                            fold_q(FOLD_G[gi])
                        MT2 = ps_mt2.tile([P, 2 * KS], f32, tag="MT2",
                                          name="MT2")
                        for j in range(NS2):
                            stat = Scur[j][:, :, kb * P:(kb + 1) * P]
                            for h in range(2):
                                m0 = pp_ * 1024 + h * KS
                                nc.tensor.matmul(
                                    MT2[:, h * KS:(h + 1) * KS], stat,
                                    vsT[j][:, :, m0:m0 + KS],
                                    start=(j == 0), stop=(j == NS2 - 1),
                                    perf_mode=DR)
                            if j == 5 and pend:
                                flush_one()
                            if j == 11 and pend:
                                flush_one()
                        Et2 = expp.tile([P, 2 * KS], bf16, tag="E",
                                        name="Et2")
                        nc.scalar.activation(Et2[:], MT2[:], AF.Exp,
                                             bias=nshift[:, 0:1], scale=0.5)
                        for h in range(2):
                            pend.append((ksub, pp_ * 2 + h, Et2, h))

            if DEBUG:
                nc.sync.dma_start(d_dbg_u[:], uT[:])
                nc.sync.dma_start(d_dbg_rest[:], restT[:])

            while pend:
                flush_one()

            if DEBUG:
                nc.sync.dma_start(d_dbg_xeT[:], xeT[:])

            # ---------- epilogue: fT = tanh(restT + xeT[:F]/l) ----------
            # l row -> (128,512) per chunk via K=1 ones matmul (f32 moving),
            # then DVE reciprocal (full-partition, ~0.3us/chunk).
            for q in range(4):
                sl = slice(q * KS, (q + 1) * KS)
                lps = ps_sh.tile([P, KS], f32, tag="sh", name="lps")
                nc.tensor.matmul(lps[:], ones1f[F:F + 1, :], xeT[F:F + 1, sl],
                                 start=True, stop=True)
                linv = bcp.tile([P, KS], f32, tag="bcf", name="linv")
                nc.vector.reciprocal_approx_fast(linv[:], lps[:])
                xf = workp.tile([F, KS], f32, tag="fin", name="xf")
                nc.vector.tensor_tensor(xf[:], xeT[:F, sl], linv[:F, :],
                                        op=ALU.mult)
                nc.vector.tensor_tensor(xf[:], xf[:], restT[:, sl],
                                        op=ALU.add)
                nc.scalar.activation(xf[:], xf[:], AF.Tanh)
                nc.sync.dma_start(d_out[:, sl], xf[:])

    nc.compile()
    return nc


def _in_maps(x, x0, alpha, beta, w, d, w1, w2, vs, bs, node_emb, conv_w,
             conv_b):
    bfl = ml_dtypes.bfloat16
    f8 = ml_dtypes.float8_e4m3
    embT = np.ascontiguousarray(node_emb.T).astype(bfl)
    cvv = np.array([[conv_w[0], conv_b[0]]], dtype=np.float32)
    bs16 = np.ascontiguousarray(bs).astype(bfl)
    Wsb = ((w * np.clip(d, 0.0, 1.0)) @ w.T).astype(np.float32)
    maps = []
    for c in range(8):
        b, h = c // 2, c % 2
        rows = slice(h * MH, (h + 1) * MH)
        xb = x[b]
        xbT = np.ascontiguousarray(xb.T)
        e1 = (xb @ w1).astype(np.float32)
        e2 = (xb @ w2).astype(np.float32)
        maps.append({
            "xb": np.ascontiguousarray(xb).astype(bfl),
            "e2b": np.ascontiguousarray(
                np.broadcast_to(e2.astype(bfl)[None, :], (P, N))),
            "e1c": np.ascontiguousarray(e1.reshape(XT, P).T),
            "Wsb": Wsb,
            "xhT": np.ascontiguousarray(xbT[:, rows]),
            "x0T": np.ascontiguousarray(x0[b].T[:, rows]),
            "alr": np.ascontiguousarray(alpha[rows])[None, :].astype(bfl),
            "ber": np.ascontiguousarray(beta[rows])[None, :].astype(bfl),
            "conv2": cvv,
            "vs8": np.ascontiguousarray(vs[rows].T).astype(f8),
            "bs16": bs16,
            "embT": embT,
            "emb_hT": np.ascontiguousarray(node_emb[rows].T).astype(bfl),
        })
    return maps


def kernel(**inputs):
    inputs = {k: np.asarray(v) for k, v in inputs.items()}
    x = inputs["x"].astype(np.float32)
    if "nc" not in _CACHE:
        _CACHE["nc"] = build_nc()
    nc = _CACHE["nc"]
    maps = _in_maps(
        x, inputs["x0"].astype(np.float32), inputs["alpha"].astype(np.float32),
        inputs["beta"].astype(np.float32), inputs["w"].astype(np.float32),
        inputs["d"].astype(np.float32), inputs["w1"].astype(np.float32),
        inputs["w2"].astype(np.float32), inputs["vs"].astype(np.float32),
        inputs["bs"].astype(np.float32), inputs["node_emb"].astype(np.float32),
        inputs["conv_w"].astype(np.float32),
        inputs["conv_b"].astype(np.float32))
    res = run_bass_kernel_spmd(nc, maps, core_ids=list(range(8)))
    out = np.empty((B, N, F), dtype=np.float32)
    for c in range(8):
        b, h = c // 2, c % 2
        out[b, h * MH:(h + 1) * MH] = np.asarray(res.results[c]["out"]).T
    return out


# revision 27
# speedup vs baseline: 1.0072x; 1.0072x over previous
"""Trainium2 Bass kernel for AGCNODEFunc (gnn_message_passing).

f = tanh(xe + 0.5*a*xa + x@W + x0*sig(beta) - 3x) where
  adj = softmax(relu(emb@emb.T), axis=1); xa = cw*(adj@x)+cb
  S[n,k] = sigmoid(e1[n]e2[k] + bs[n,k]); M = vs@S; Emat = softmax(M, -1); xe = Emat@x

Sharding: 8 cores = 4 batches x 2 row-halves (fully data-parallel).

v4: everything computed TRANSPOSED (no PE transposes); the N^3 matmul in
fp8 DoubleRow. Relative to v3:
  - phase A (adj@x: z = emb@emb^T, exp, u = [x|1]^T @ exp(relu(z))) is
    INTERLEAVED into the main MT sweep at accumulation-group boundaries,
    so the PE never idles and HAM stays at K=8/8 (v3 ran the whole phase
    at half clock: 192us of K=4/8).
  - MT PSUM pairs into one (128,1024) tile spanning 2 banks -> ONE
    1024-wide exp per (kb,pp) group ((N+352)/1.2 scalar cost amortized).
  - S' arg planes split: i=0 on DVE, i=1 on GpSimd.
  - bs/vs DMAs consolidated to one (128,2,*) DMA per pair-tile; x tiles
    for the xe/u stationaries land in ONE 512KB DMA (host pre-casts bf16).
  - xa fold + epilogue use broadcast-FIRST reciprocals ((128,2048) DVE
    reciprocal is 1us; v3's single-lane (1,2048) was 15.7us) and the
    softmax denominator row is broadcast by a K=1 ones matmul on the PE
    (v3 did two DRAM round-trips).
Softmax over k needs NO max pass: logits 0.5*M lie in [-140, 140], so
exp(0.5*MT - 64) neither overflows f32 nor flushes whole rows.
Output (F, MH) transposed; host transposes back.
"""

import numpy as np
import ml_dtypes

import concourse.bass as bass
import concourse.bacc as bacc
import concourse.mybir as mybir
from concourse import tile
from concourse.bass_utils import run_bass_kernel_spmd

B, N, F, E = 4, 4096, 64, 16
P = 128
MH = N // 2            # 2048 m-rows per core
KS = 512               # k-strip width
NSTR = N // KS         # 8 strips
NS2 = N // 256         # 16 pair-subtiles over n (contraction)
XT = N // P            # 32 x tiles
CSH = 64.0             # softmax constant shift (see module docstring)
f32 = mybir.dt.float32
bf16 = mybir.dt.bfloat16
fp8 = mybir.dt.float8e4
AF = mybir.ActivationFunctionType
ALU = mybir.AluOpType
DR = mybir.MatmulPerfMode.DoubleRow

_CACHE = {}
DEBUG = False


def build_nc():
    nc = bacc.Bacc()
    d_xb = nc.dram_tensor("xb", (N, F), bf16, kind="ExternalInput")
    d_e2b = nc.dram_tensor("e2b", (P, N), bf16, kind="ExternalInput")
    d_e1c = nc.dram_tensor("e1c", (P, XT), f32, kind="ExternalInput")
    d_Wsb = nc.dram_tensor("Wsb", (F, F), f32, kind="ExternalInput")
    d_xhT = nc.dram_tensor("xhT", (F, MH), f32, kind="ExternalInput")
    d_x0T = nc.dram_tensor("x0T", (F, MH), f32, kind="ExternalInput")
    d_alr = nc.dram_tensor("alr", (1, MH), bf16, kind="ExternalInput")
    d_ber = nc.dram_tensor("ber", (1, MH), bf16, kind="ExternalInput")
    d_cv = nc.dram_tensor("conv2", (1, 2), f32, kind="ExternalInput")
    d_vs8 = nc.dram_tensor("vs8", (N, MH), fp8, kind="ExternalInput")
    d_bs16 = nc.dram_tensor("bs16", (N, N), bf16, kind="ExternalInput")
    d_embT = nc.dram_tensor("embT", (E, N), bf16, kind="ExternalInput")
    d_embhT = nc.dram_tensor("emb_hT", (E, MH), bf16, kind="ExternalInput")
    d_out = nc.dram_tensor("out", (F, MH), f32, kind="ExternalOutput")
    if DEBUG:
        d_dbg_e2b = nc.dram_tensor("dbg_e2b", (P, N), bf16,
                                   kind="ExternalOutput")
        d_dbg_e12 = nc.dram_tensor("dbg_e12", (P, 2 * XT), f32,
                                   kind="ExternalOutput")
        d_dbg_u = nc.dram_tensor("dbg_u", (F + 1, MH), f32,
                                 kind="ExternalOutput")
        d_dbg_xeT = nc.dram_tensor("dbg_xeT", (F + 1, MH), f32,
                                   kind="ExternalOutput")
        d_dbg_rest = nc.dram_tensor("dbg_rest", (F, MH), f32,
                                    kind="ExternalOutput")
        d_dbg_xeb = nc.dram_tensor("dbg_xeb", (P, XT * (F + 1)), bf16,
                                   kind="ExternalOutput")

    with tile.TileContext(nc) as tc:
        with (
            tc.tile_pool(name="persist", bufs=1) as persist,
            tc.tile_pool(name="vspool", bufs=1) as vspool,
            tc.tile_pool(name="spool", bufs=1) as spool,
            tc.tile_pool(name="bsq", bufs=2) as bsqp,
            tc.tile_pool(name="work", bufs=3) as workp,
            tc.tile_pool(name="exp", bufs=3) as expp,
            tc.tile_pool(name="ez", bufs=7) as ezp,
            tc.tile_pool(name="rows", bufs=4) as rowsp,
            tc.tile_pool(name="bcast", bufs=2) as bcp,
            tc.tile_pool(name="xrot", bufs=2) as xrot,
            # PSUM: mt2 2x2 banks + shared 3 + ups 1 = 8 banks
            tc.tile_pool(name="ps_mt2", bufs=2, space="PSUM") as ps_mt2,
            tc.tile_pool(name="ps_sh", bufs=3, space="PSUM") as ps_sh,
            tc.tile_pool(name="ps_ups", bufs=1, space="PSUM") as ps_ups,
        ):
            # ---------- persistent tiles ----------
            e2b = persist.tile([P, N], bf16)          # e2 bcast over partitions
            nshift = persist.tile([P, 1], f32)        # exp bias = -CSH
            nc.vector.memset(nshift[:], -CSH)
            ones1 = persist.tile([1, P], bf16)        # K=1 bcast stationary
            nc.vector.memset(ones1[:], 1.0)
            ones1f = persist.tile([F + 1, P], f32)    # row F used (part. 64)
            nc.vector.memset(ones1f[F:F + 1, :], 1.0)
            e1c = persist.tile([P, XT], f32)          # col j = e1 of n-block j
            cv = persist.tile([1, 2], f32)
            nc.sync.dma_start(cv[:], d_cv[:])
            cvb = persist.tile([P, 2], f32)
            nc.gpsimd.partition_broadcast(cvb[:], cv[:])
            # stationaries for xe/u matmuls: [x|1] bf16, (128, 32, 65)
            xeb = persist.tile([P, XT, F + 1], bf16)
            restT = persist.tile([F, MH], f32)
            xeT = persist.tile([F + 1, MH], f32)
            uT = persist.tile([F + 1, MH], f32)
            embT = persist.tile([E, N], bf16)
            embhT = persist.tile([E, MH], bf16)
            # vs^T fp8 pair tiles: vsT[j][p, i, m] = vs[m, j*256 + i*128 + p]
            vsT = [vspool.tile([P, 2, MH], fp8, tag=f"vsT{j}", name=f"vsT{j}")
                   for j in range(NS2)]
            # S' fp8 double-buffered strip tiles
            S8 = [[spool.tile([P, 2, KS], fp8, tag=f"S{par}_{j}",
                              name=f"S{par}_{j}") for j in range(NS2)]
                  for par in range(2)]

            # ---------- head DMAs (small first) ----------
            nc.sync.dma_start(embT[:], d_embT[:])
            nc.sync.dma_start(embhT[:], d_embhT[:])
            Wsb = persist.tile([F, F], f32)
            nc.sync.dma_start(Wsb[:], d_Wsb[:])
            nc.sync.dma_start(e1c[:], d_e1c[:])
            alr = rowsp.tile([1, MH], bf16, tag="row", name="alr")
            nc.sync.dma_start(alr[:], d_alr[:])
            ber = rowsp.tile([1, MH], bf16, tag="row", name="ber")
            nc.sync.dma_start(ber[:], d_ber[:])
            # all 32 x tiles in one DMA; ones column via strided memset
            nc.sync.dma_start(xeb[:, :, :F],
                              d_xb[:].rearrange("(k p) f -> p k f", p=P))
            nc.vector.memset(xeb[:, :, F:F + 1], 1.0)
            nc.sync.dma_start(e2b[:], d_e2b[:])

            # ---------- strip production: S' = tanh(0.5(e1 e2^T + bs)) ------
            def produce_j(s, j):
                par = s % 2
                k0 = s * KS
                if True:
                    bsq = bsqp.tile([P, 2, KS], bf16, tag="bsq", name="bsq")
                    nc.sync.dma_start(
                        bsq[:],
                        d_bs16[j * 256:(j + 1) * 256, k0:k0 + KS]
                        .rearrange("(i p) k -> p i k", p=P))
                    arg = workp.tile([P, 2, KS], bf16, tag="arg", name="arg")
                    for i, eng in ((0, nc.vector), (1, nc.vector)):
                        eng.scalar_tensor_tensor(
                            arg[:, i, :], e2b[:, k0:k0 + KS],
                            e1c[:, (2 * j + i):(2 * j + i) + 1],
                            bsq[:, i, :], op0=ALU.mult, op1=ALU.add)
                    nc.scalar.activation(S8[par][j][:], arg[:], AF.Tanh,
                                         scale=0.5)

            def produce(s):
                for j in range(NS2):
                    produce_j(s, j)

            produce(0)

            # vs8 ahead of restT inputs (needed by sweep group 0)
            for j in range(NS2):
                nc.sync.dma_start(
                    vsT[j][:],
                    d_vs8[j * 256:(j + 1) * 256, :]
                    .rearrange("(i p) m -> p i m", p=P))

            # ---------- restT = xw^T + x0^T*sig(beta) - 3x^T ----------
            sbr = rowsp.tile([1, MH], bf16, tag="row", name="sbr")
            nc.scalar.activation(sbr[:], ber[:], AF.Sigmoid)
            REST_G = {1: 0, 3: 1, 5: 2, 7: 3}

            def rest_q(q):
                sl = slice(q * KS, (q + 1) * KS)
                # broadcast sig(beta) chunk via K=1 ones matmul (gpsimd
                # partition_broadcast mishandles src free-offsets on HW)
                sbps = ps_sh.tile([P, KS], f32, tag="sh", name="sbps")
                nc.tensor.matmul(sbps[:], ones1[:], sbr[:, sl],
                                 start=True, stop=True)
                xhc = xrot.tile([F, KS], f32, tag="xc", name="xhc")
                nc.sync.dma_start(xhc[:], d_xhT[:, sl])
                x0c = xrot.tile([F, KS], f32, tag="x0c", name="x0c")
                nc.sync.dma_start(x0c[:], d_x0T[:, sl])
                xwps = ps_sh.tile([P, KS], f32, tag="sh", name="xwps")
                nc.tensor.matmul(xwps[:F, :], Wsb[:], xhc[:],
                                 start=True, stop=True)
                nc.vector.scalar_tensor_tensor(
                    restT[:, sl], xhc[:], -3.0, xwps[:F, :],
                    op0=ALU.mult, op1=ALU.add)
                t0 = workp.tile([F, KS], f32, tag="fin", name="t0")
                nc.vector.tensor_tensor(t0[:], x0c[:], sbps[:F, :],
                                        op=ALU.mult)
                nc.vector.tensor_tensor(restT[:, sl], restT[:, sl], t0[:],
                                        op=ALU.add)

            nc.vector.memset(xeT[:], 0.0)

            # ---------- phase A ops interleaved into the sweep ----------
            # z[i]: zps = embT_ns^T @ embh_mb ; ez = max(exp(zps),1)
            # u[i]: ups_mb += [x|1]_ns^T @ ez   (32 accumulating MMs per mb)
            NPA = 4 * XT                         # 128 z ops / 128 u ops
            PA_G0, PA_G1 = 8, 58                 # groups of strips 1..7
            z_sched = {}
            u_sched = {}
            for i in range(NPA):
                g = PA_G0 + (i * (PA_G1 - PA_G0)) // NPA
                z_sched.setdefault(g, []).append(i)
                u_sched.setdefault(g + 2, []).append(i)
            ez_buf = {}
            ups_cur = [None]

            def pa_zu(gi):
                for i in u_sched.get(gi, ()):
                    mb, ns = i // XT, i % XT
                    if ns == 0:
                        ups_cur[0] = ps_ups.tile([F + 1, KS], f32, tag="UPS",
                                                 name="upsT")
                    nc.tensor.matmul(ups_cur[0][:], xeb[:, ns, :],
                                     ez_buf.pop(i)[:],
                                     start=(ns == 0), stop=(ns == XT - 1))
                    if ns == XT - 1:
                        nc.vector.tensor_copy(
                            uT[:, mb * KS:(mb + 1) * KS], ups_cur[0][:])
                for i in z_sched.get(gi, ()):
                    mb, ns = i // XT, i % XT
                    zps = ps_sh.tile([P, KS], f32, tag="sh", name="zps")
                    nc.tensor.matmul(zps[:], embT[:, ns * P:(ns + 1) * P],
                                     embhT[:, mb * KS:(mb + 1) * KS],
                                     start=True, stop=True)
                    ez = ezp.tile([P, KS], bf16, tag="ez", name="ez")
                    nc.scalar.activation(ez[:], zps[:], AF.Exp)
                    nc.vector.tensor_scalar_max(ez[:], ez[:], 1.0)
                    ez_buf[i] = ez

            # fold xa chunk q: rest += (0.5*sa*cw/urow)*u[:F] + 0.5*sa*cb
            # (chunk q only needs mb=q's u columns; interleaved into strip 7)
            sar = rowsp.tile([1, MH], bf16, tag="row", name="sar")
            nc.scalar.activation(sar[:], alr[:], AF.Sigmoid)
            FOLD_G = {28: 0, 42: 1, 54: 2, 60: 3}

            def fold_q(q):
                sl = slice(q * KS, (q + 1) * KS)
                saps = ps_sh.tile([P, KS], f32, tag="sh", name="saps")
                nc.tensor.matmul(saps[:], ones1[:], sar[:, sl],
                                 start=True, stop=True)
                urps = ps_sh.tile([P, KS], f32, tag="sh", name="urps")
                nc.tensor.matmul(urps[:], ones1f[F:F + 1, :],
                                 uT[F:F + 1, sl], start=True, stop=True)
                s1b = bcp.tile([P, KS], f32, tag="bcf", name="s1b")
                nc.vector.reciprocal_approx_fast(s1b[:], urps[:])
                nc.vector.tensor_tensor(s1b[:], saps[:], s1b[:],
                                        op=ALU.mult)
                nc.vector.tensor_scalar(s1b[:], s1b[:], cvb[:, 0:1], 0.5,
                                        op0=ALU.mult, op1=ALU.mult)
                s0b = bcp.tile([P, KS], bf16, tag="bc", name="s0b")
                nc.vector.tensor_scalar(s0b[:], saps[:], cvb[:, 1:2], 0.5,
                                        op0=ALU.mult, op1=ALU.mult)
                t1 = workp.tile([F, KS], f32, tag="fin", name="t1")
                nc.vector.tensor_tensor(t1[:], uT[:F, sl], s1b[:F, :],
                                        op=ALU.mult)
                nc.vector.tensor_tensor(t1[:], t1[:], s0b[:F, :],
                                        op=ALU.add)
                nc.vector.tensor_tensor(restT[:, sl], restT[:, sl],
                                        t1[:], op=ALU.add)

            # ---------- main sweep: MT = S'^T vs^T (fp8 DoubleRow) ----------
            pend = []                # FIFO of (ksub, q4, Et2, h)

            def flush_one():
                ksub, q4, Et2, h = pend.pop(0)
                xeps = ps_sh.tile([F + 1, KS], f32, tag="sh", name="xeps")
                nc.tensor.matmul(xeps[:], xeb[:, ksub, :],
                                 Et2[:, h * KS:(h + 1) * KS],
                                 start=True, stop=True)
                nc.vector.tensor_tensor(
                    xeT[:, q4 * KS:(q4 + 1) * KS],
                    xeT[:, q4 * KS:(q4 + 1) * KS], xeps[:], op=ALU.add)

            for s in range(NSTR):
                Scur = S8[s % 2]
                for kb in range(4):
                    ksub = 4 * s + kb
                    for pp_ in range(2):
                        gi = s * 8 + kb * 2 + pp_
                        gl = kb * 2 + pp_
                        if s < NSTR - 1:
                            produce_j(s + 1, 2 * gl)
                            produce_j(s + 1, 2 * gl + 1)
                        pa_zu(gi)
                        if gi in REST_G:
                            rest_q(REST_G[gi])
                        if gi in FOLD_G:
                            fold_q(FOLD_G[gi])
                        MT2 = ps_mt2.tile([P, 2 * KS], f32, tag="MT2",
                                          name="MT2")
                        for j in range(NS2):
                            stat = Scur[j][:, :, kb * P:(kb + 1) * P]
                            for h in range(2):
                                m0 = pp_ * 1024 + h * KS
                                nc.tensor.matmul(
                                    MT2[:, h * KS:(h + 1) * KS], stat,
                                    vsT[j][:, :, m0:m0 + KS],
                                    start=(j == 0), stop=(j == NS2 - 1),
                                    perf_mode=DR)
                            if j == 5 and pend:
                                flush_one()
                            if j == 11 and pend:
                                flush_one()
                        Et2 = expp.tile([P, 2 * KS], bf16, tag="E",
                                        name="Et2")
                        nc.scalar.activation(Et2[:], MT2[:], AF.Exp,
                                             bias=nshift[:, 0:1], scale=0.5)
                        for h in range(2):
                            pend.append((ksub, pp_ * 2 + h, Et2, h))

            if DEBUG:
                nc.sync.dma_start(d_dbg_u[:], uT[:])
                nc.sync.dma_start(d_dbg_rest[:], restT[:])

            while pend:
                flush_one()

            if DEBUG:
                nc.sync.dma_start(d_dbg_xeT[:], xeT[:])

            # ---------- epilogue: fT = tanh(restT + xeT[:F]/l) ----------
            # l row -> (128,512) per chunk via K=1 ones matmul (f32 moving),
            # then DVE reciprocal (full-partition, ~0.3us/chunk).
            for q in range(4):
                sl = slice(q * KS, (q + 1) * KS)
                lps = ps_sh.tile([P, KS], f32, tag="sh", name="lps")
                nc.tensor.matmul(lps[:], ones1f[F:F + 1, :], xeT[F:F + 1, sl],
                                 start=True, stop=True)
                linv = bcp.tile([P, KS], f32, tag="bcf", name="linv")
                nc.vector.reciprocal_approx_fast(linv[:], lps[:])
                xf = workp.tile([F, KS], f32, tag="fin", name="xf")
                nc.vector.tensor_tensor(xf[:], xeT[:F, sl], linv[:F, :],
                                        op=ALU.mult)
                nc.vector.tensor_tensor(xf[:], xf[:], restT[:, sl],
                                        op=ALU.add)
                nc.scalar.activation(xf[:], xf[:], AF.Tanh)
                nc.sync.dma_start(d_out[:, sl], xf[:])

    nc.compile()
    return nc


def _in_maps(x, x0, alpha, beta, w, d, w1, w2, vs, bs, node_emb, conv_w,
             conv_b):
    bfl = ml_dtypes.bfloat16
    f8 = ml_dtypes.float8_e4m3
    embT = np.ascontiguousarray(node_emb.T).astype(bfl)
    cvv = np.array([[conv_w[0], conv_b[0]]], dtype=np.float32)
    bs16 = np.ascontiguousarray(bs).astype(bfl)
    Wsb = ((w * np.clip(d, 0.0, 1.0)) @ w.T).astype(np.float32)
    maps = []
    for c in range(8):
        b, h = c // 2, c % 2
        rows = slice(h * MH, (h + 1) * MH)
        xb = x[b]
        xbT = np.ascontiguousarray(xb.T)
        e1 = (xb @ w1).astype(np.float32)
        e2 = (xb @ w2).astype(np.float32)
        maps.append({
            "xb": np.ascontiguousarray(xb).astype(bfl),
            "e2b": np.ascontiguousarray(
                np.broadcast_to(e2.astype(bfl)[None, :], (P, N))),
            "e1c": np.ascontiguousarray(e1.reshape(XT, P).T),
            "Wsb": Wsb,
            "xhT": np.ascontiguousarray(xbT[:, rows]),
            "x0T": np.ascontiguousarray(x0[b].T[:, rows]),
            "alr": np.ascontiguousarray(alpha[rows])[None, :].astype(bfl),
            "ber": np.ascontiguousarray(beta[rows])[None, :].astype(bfl),
            "conv2": cvv,
            "vs8": np.ascontiguousarray(vs[rows].T).astype(f8),
            "bs16": bs16,
            "embT": embT,
            "emb_hT": np.ascontiguousarray(node_emb[rows].T).astype(bfl),
        })
    return maps


def kernel(**inputs):
    inputs = {k: np.asarray(v) for k, v in inputs.items()}
    x = inputs["x"].astype(np.float32)
    if "nc" not in _CACHE:
        _CACHE["nc"] = build_nc()
    nc = _CACHE["nc"]
    maps = _in_maps(
        x, inputs["x0"].astype(np.float32), inputs["alpha"].astype(np.float32),
        inputs["beta"].astype(np.float32), inputs["w"].astype(np.float32),
        inputs["d"].astype(np.float32), inputs["w1"].astype(np.float32),
        inputs["w2"].astype(np.float32), inputs["vs"].astype(np.float32),
        inputs["bs"].astype(np.float32), inputs["node_emb"].astype(np.float32),
        inputs["conv_w"].astype(np.float32),
        inputs["conv_b"].astype(np.float32))
    res = run_bass_kernel_spmd(nc, maps, core_ids=list(range(8)))
    out = np.empty((B, N, F), dtype=np.float32)
    for c in range(8):
        b, h = c // 2, c % 2
        out[b, h * MH:(h + 1) * MH] = np.asarray(res.results[c]["out"]).T
    return out


# revision 28
# speedup vs baseline: 1.0288x; 1.0214x over previous
"""Trainium2 Bass kernel for AGCNODEFunc (gnn_message_passing).

f = tanh(xe + 0.5*a*xa + x@W + x0*sig(beta) - 3x) where
  adj = softmax(relu(emb@emb.T), axis=1); xa = cw*(adj@x)+cb
  S[n,k] = sigmoid(e1[n]e2[k] + bs[n,k]); M = vs@S; Emat = softmax(M, -1); xe = Emat@x

Sharding: 8 cores = 4 batches x 2 row-halves (fully data-parallel).

v4: everything computed TRANSPOSED (no PE transposes); the N^3 matmul in
fp8 DoubleRow. Relative to v3:
  - phase A (adj@x: z = emb@emb^T, exp, u = [x|1]^T @ exp(relu(z))) is
    INTERLEAVED into the main MT sweep at accumulation-group boundaries,
    so the PE never idles and HAM stays at K=8/8 (v3 ran the whole phase
    at half clock: 192us of K=4/8).
  - MT PSUM pairs into one (128,1024) tile spanning 2 banks -> ONE
    1024-wide exp per (kb,pp) group ((N+352)/1.2 scalar cost amortized).
  - S' arg planes split: i=0 on DVE, i=1 on GpSimd.
  - bs/vs DMAs consolidated to one (128,2,*) DMA per pair-tile; x tiles
    for the xe/u stationaries land in ONE 512KB DMA (host pre-casts bf16).
  - xa fold + epilogue use broadcast-FIRST reciprocals ((128,2048) DVE
    reciprocal is 1us; v3's single-lane (1,2048) was 15.7us) and the
    softmax denominator row is broadcast by a K=1 ones matmul on the PE
    (v3 did two DRAM round-trips).
Softmax over k needs NO max pass: logits 0.5*M lie in [-140, 140], so
exp(0.5*MT - 64) neither overflows f32 nor flushes whole rows.
Output (F, MH) transposed; host transposes back.
"""

import numpy as np
import ml_dtypes

import concourse.bass as bass
import concourse.bacc as bacc
import concourse.mybir as mybir
from concourse import tile
from concourse.bass_utils import run_bass_kernel_spmd

B, N, F, E = 4, 4096, 64, 16
P = 128
MH = N // 2            # 2048 m-rows per core
KS = 512               # k-strip width
NSTR = N // KS         # 8 strips
NS2 = N // 256         # 16 pair-subtiles over n (contraction)
XT = N // P            # 32 x tiles
CSH = 64.0             # softmax constant shift (see module docstring)
f32 = mybir.dt.float32
bf16 = mybir.dt.bfloat16
fp8 = mybir.dt.float8e4
AF = mybir.ActivationFunctionType
ALU = mybir.AluOpType
DR = mybir.MatmulPerfMode.DoubleRow

_CACHE = {}
DEBUG = False


def build_nc():
    nc = bacc.Bacc()
    d_xb = nc.dram_tensor("xb", (N, F), bf16, kind="ExternalInput")
    d_e2b = nc.dram_tensor("e2b", (P, N), bf16, kind="ExternalInput")
    d_e1c = nc.dram_tensor("e1c", (P, XT), f32, kind="ExternalInput")
    d_Wsb = nc.dram_tensor("Wsb", (F, F), f32, kind="ExternalInput")
    d_xhT = nc.dram_tensor("xhT", (F, MH), f32, kind="ExternalInput")
    d_x0T = nc.dram_tensor("x0T", (F, MH), f32, kind="ExternalInput")
    d_alr = nc.dram_tensor("alr", (1, MH), bf16, kind="ExternalInput")
    d_ber = nc.dram_tensor("ber", (1, MH), bf16, kind="ExternalInput")
    d_cv = nc.dram_tensor("conv2", (1, 2), f32, kind="ExternalInput")
    d_vs8 = nc.dram_tensor("vs8", (N, MH), fp8, kind="ExternalInput")
    d_bs16 = nc.dram_tensor("bs16", (N, N), bf16, kind="ExternalInput")
    d_embT = nc.dram_tensor("embT", (E, N), bf16, kind="ExternalInput")
    d_embhT = nc.dram_tensor("emb_hT", (E, MH), bf16, kind="ExternalInput")
    d_out = nc.dram_tensor("out", (F, MH), f32, kind="ExternalOutput")
    if DEBUG:
        d_dbg_e2b = nc.dram_tensor("dbg_e2b", (P, N), bf16,
                                   kind="ExternalOutput")
        d_dbg_e12 = nc.dram_tensor("dbg_e12", (P, 2 * XT), f32,
                                   kind="ExternalOutput")
        d_dbg_u = nc.dram_tensor("dbg_u", (F + 1, MH), f32,
                                 kind="ExternalOutput")
        d_dbg_xeT = nc.dram_tensor("dbg_xeT", (F + 1, MH), f32,
                                   kind="ExternalOutput")
        d_dbg_rest = nc.dram_tensor("dbg_rest", (F, MH), f32,
                                    kind="ExternalOutput")
        d_dbg_xeb = nc.dram_tensor("dbg_xeb", (P, XT * (F + 1)), bf16,
                                   kind="ExternalOutput")

    with tile.TileContext(nc) as tc:
        with (
            tc.tile_pool(name="persist", bufs=1) as persist,
            tc.tile_pool(name="vspool", bufs=1) as vspool,
            tc.tile_pool(name="spool", bufs=1) as spool,
            tc.tile_pool(name="bsq", bufs=4) as bsqp,
            tc.tile_pool(name="work", bufs=3) as workp,
            tc.tile_pool(name="exp", bufs=3) as expp,
            tc.tile_pool(name="ez", bufs=7) as ezp,
            tc.tile_pool(name="rows", bufs=3) as rowsp,
            tc.tile_pool(name="bcast", bufs=2) as bcp,
            tc.tile_pool(name="xrot", bufs=2) as xrot,
            # PSUM: mt2 2x2 banks + shared 3 + ups 1 = 8 banks
            tc.tile_pool(name="ps_mt2", bufs=2, space="PSUM") as ps_mt2,
            tc.tile_pool(name="ps_sh", bufs=3, space="PSUM") as ps_sh,
            tc.tile_pool(name="ps_ups", bufs=1, space="PSUM") as ps_ups,
        ):
            # ---------- persistent tiles ----------
            e2b = persist.tile([P, N], bf16)          # e2 bcast over partitions
            nshift = persist.tile([P, 1], f32)        # exp bias = -CSH
            nc.vector.memset(nshift[:], -CSH)
            ones1 = persist.tile([1, P], bf16)        # K=1 bcast stationary
            nc.vector.memset(ones1[:], 1.0)
            ones1f = persist.tile([F + 1, P], f32)    # row F used (part. 64)
            nc.vector.memset(ones1f[F:F + 1, :], 1.0)
            e1c = persist.tile([P, XT], f32)          # col j = e1 of n-block j
            cv = persist.tile([1, 2], f32)
            nc.sync.dma_start(cv[:], d_cv[:])
            cvb = persist.tile([P, 2], f32)
            nc.gpsimd.partition_broadcast(cvb[:], cv[:])
            # stationaries for xe/u matmuls: [x|1] bf16, (128, 32, 65)
            xeb = persist.tile([P, XT, F + 1], bf16)
            restT = persist.tile([F, MH], f32)
            xeT = persist.tile([F + 1, MH], f32)
            uT = persist.tile([F + 1, MH], f32)
            embT = persist.tile([E, N], bf16)
            embhT = persist.tile([E, MH], bf16)
            # vs^T fp8 pair tiles: vsT[j][p, i, m] = vs[m, j*256 + i*128 + p]
            vsT = [vspool.tile([P, 2, MH], fp8, tag=f"vsT{j}", name=f"vsT{j}")
                   for j in range(NS2)]
            # S' fp8 double-buffered strip tiles
            S8 = [[spool.tile([P, 2, KS], fp8, tag=f"S{par}_{j}",
                              name=f"S{par}_{j}") for j in range(NS2)]
                  for par in range(2)]

            # ---------- head DMAs (small first) ----------
            nc.sync.dma_start(embT[:], d_embT[:])
            nc.sync.dma_start(embhT[:], d_embhT[:])
            Wsb = persist.tile([F, F], f32)
            nc.sync.dma_start(Wsb[:], d_Wsb[:])
            nc.sync.dma_start(e1c[:], d_e1c[:])
            alr = rowsp.tile([1, MH], bf16, tag="row", name="alr")
            nc.sync.dma_start(alr[:], d_alr[:])
            ber = rowsp.tile([1, MH], bf16, tag="row", name="ber")
            nc.sync.dma_start(ber[:], d_ber[:])
            sar = rowsp.tile([1, MH], bf16, tag="row", name="sar")
            nc.scalar.activation(sar[:], alr[:], AF.Sigmoid)
            sbr = rowsp.tile([1, MH], bf16, tag="row", name="sbr")
            nc.scalar.activation(sbr[:], ber[:], AF.Sigmoid)
            # all 32 x tiles in one DMA; ones column via strided memset
            for c in range(4):
                nc.sync.dma_start(
                    e2b[:, c * N // 4:(c + 1) * N // 4],
                    d_e2b[:, c * N // 4:(c + 1) * N // 4])
                nc.sync.dma_start(
                    xeb[:, c * 8:(c + 1) * 8, :F],
                    d_xb[c * 1024:(c + 1) * 1024]
                    .rearrange("(k p) f -> p k f", p=P))
            nc.vector.memset(xeb[:, :, F:F + 1], 1.0)

            # ---------- strip production: S' = tanh(0.5(e1 e2^T + bs)) ------
            def produce_j(s, j):
                par = s % 2
                k0 = s * KS
                if True:
                    bsq = bsqp.tile([P, 2, KS], bf16, tag="bsq", name="bsq")
                    nc.sync.dma_start(
                        bsq[:],
                        d_bs16[j * 256:(j + 1) * 256, k0:k0 + KS]
                        .rearrange("(i p) k -> p i k", p=P))
                    arg = workp.tile([P, 2, KS], bf16, tag="arg", name="arg")
                    for i, eng in ((0, nc.vector), (1, nc.vector)):
                        eng.scalar_tensor_tensor(
                            arg[:, i, :], e2b[:, k0:k0 + KS],
                            e1c[:, (2 * j + i):(2 * j + i) + 1],
                            bsq[:, i, :], op0=ALU.mult, op1=ALU.add)
                    nc.scalar.activation(S8[par][j][:], arg[:], AF.Tanh,
                                         scale=0.5)

            def produce(s):
                for j in range(NS2):
                    produce_j(s, j)

            produce(0)

            # vs8 ahead of restT inputs (needed by sweep group 0)
            for j in range(NS2):
                nc.sync.dma_start(
                    vsT[j][:],
                    d_vs8[j * 256:(j + 1) * 256, :]
                    .rearrange("(i p) m -> p i m", p=P))

            # ---------- restT = xw^T + x0^T*sig(beta) - 3x^T ----------
            REST_G = {1: 0, 3: 1, 5: 2, 7: 3}

            def rest_q(q):
                sl = slice(q * KS, (q + 1) * KS)
                # broadcast sig(beta) chunk via K=1 ones matmul (gpsimd
                # partition_broadcast mishandles src free-offsets on HW)
                sbps = ps_sh.tile([P, KS], f32, tag="sh", name="sbps")
                nc.tensor.matmul(sbps[:], ones1[:], sbr[:, sl],
                                 start=True, stop=True)
                xhc = xrot.tile([F, KS], f32, tag="xc", name="xhc")
                nc.sync.dma_start(xhc[:], d_xhT[:, sl])
                x0c = xrot.tile([F, KS], f32, tag="x0c", name="x0c")
                nc.sync.dma_start(x0c[:], d_x0T[:, sl])
                xwps = ps_sh.tile([P, KS], f32, tag="sh", name="xwps")
                nc.tensor.matmul(xwps[:F, :], Wsb[:], xhc[:],
                                 start=True, stop=True)
                nc.vector.scalar_tensor_tensor(
                    restT[:, sl], xhc[:], -3.0, xwps[:F, :],
                    op0=ALU.mult, op1=ALU.add)
                t0 = workp.tile([F, KS], f32, tag="fin", name="t0")
                nc.vector.tensor_tensor(t0[:], x0c[:], sbps[:F, :],
                                        op=ALU.mult)
                nc.vector.tensor_tensor(restT[:, sl], restT[:, sl], t0[:],
                                        op=ALU.add)

            nc.vector.memset(xeT[:], 0.0)

            # ---------- phase A ops interleaved into the sweep ----------
            # z[i]: zps = embT_ns^T @ embh_mb ; ez = max(exp(zps),1)
            # u[i]: ups_mb += [x|1]_ns^T @ ez   (32 accumulating MMs per mb)
            NPA = 4 * XT                         # 128 z ops / 128 u ops
            PA_G0, PA_G1 = 8, 58                 # groups of strips 1..7
            z_sched = {}
            u_sched = {}
            for i in range(NPA):
                g = PA_G0 + (i * (PA_G1 - PA_G0)) // NPA
                z_sched.setdefault(g, []).append(i)
                u_sched.setdefault(g + 2, []).append(i)
            ez_buf = {}
            ups_cur = [None]

            def pa_zu(gi):
                for i in u_sched.get(gi, ()):
                    mb, ns = i // XT, i % XT
                    if ns == 0:
                        ups_cur[0] = ps_ups.tile([F + 1, KS], f32, tag="UPS",
                                                 name="upsT")
                    nc.tensor.matmul(ups_cur[0][:], xeb[:, ns, :],
                                     ez_buf.pop(i)[:],
                                     start=(ns == 0), stop=(ns == XT - 1))
                    if ns == XT - 1:
                        nc.vector.tensor_copy(
                            uT[:, mb * KS:(mb + 1) * KS], ups_cur[0][:])
                for i in z_sched.get(gi, ()):
                    mb, ns = i // XT, i % XT
                    zps = ps_sh.tile([P, KS], f32, tag="sh", name="zps")
                    nc.tensor.matmul(zps[:], embT[:, ns * P:(ns + 1) * P],
                                     embhT[:, mb * KS:(mb + 1) * KS],
                                     start=True, stop=True)
                    ez = ezp.tile([P, KS], bf16, tag="ez", name="ez")
                    nc.scalar.activation(ez[:], zps[:], AF.Exp)
                    nc.vector.tensor_scalar_max(ez[:], ez[:], 1.0)
                    ez_buf[i] = ez

            # fold xa chunk q: rest += (0.5*sa*cw/urow)*u[:F] + 0.5*sa*cb
            # (chunk q only needs mb=q's u columns; interleaved into strip 7)
            FOLD_G = {28: 0, 42: 1, 54: 2, 60: 3}

            def fold_q(q):
                sl = slice(q * KS, (q + 1) * KS)
                saps = ps_sh.tile([P, KS], f32, tag="sh", name="saps")
                nc.tensor.matmul(saps[:], ones1[:], sar[:, sl],
                                 start=True, stop=True)
                urps = ps_sh.tile([P, KS], f32, tag="sh", name="urps")
                nc.tensor.matmul(urps[:], ones1f[F:F + 1, :],
                                 uT[F:F + 1, sl], start=True, stop=True)
                s1b = bcp.tile([P, KS], f32, tag="bcf", name="s1b")
                nc.vector.reciprocal_approx_fast(s1b[:], urps[:])
                nc.vector.tensor_tensor(s1b[:], saps[:], s1b[:],
                                        op=ALU.mult)
                nc.vector.tensor_scalar(s1b[:], s1b[:], cvb[:, 0:1], 0.5,
                                        op0=ALU.mult, op1=ALU.mult)
                s0b = bcp.tile([P, KS], bf16, tag="bc", name="s0b")
                nc.vector.tensor_scalar(s0b[:], saps[:], cvb[:, 1:2], 0.5,
                                        op0=ALU.mult, op1=ALU.mult)
                t1 = workp.tile([F, KS], f32, tag="fin", name="t1")
                nc.vector.tensor_tensor(t1[:], uT[:F, sl], s1b[:F, :],
                                        op=ALU.mult)
                nc.vector.tensor_tensor(t1[:], t1[:], s0b[:F, :],
                                        op=ALU.add)
                nc.vector.tensor_tensor(restT[:, sl], restT[:, sl],
                                        t1[:], op=ALU.add)

            # ---------- main sweep: MT = S'^T vs^T (fp8 DoubleRow) ----------
            pend = []                # FIFO of (ksub, q4, Et2, h)

            def flush_one():
                ksub, q4, Et2, h = pend.pop(0)
                xeps = ps_sh.tile([F + 1, KS], f32, tag="sh", name="xeps")
                nc.tensor.matmul(xeps[:], xeb[:, ksub, :],
                                 Et2[:, h * KS:(h + 1) * KS],
                                 start=True, stop=True)
                nc.vector.tensor_tensor(
                    xeT[:, q4 * KS:(q4 + 1) * KS],
                    xeT[:, q4 * KS:(q4 + 1) * KS], xeps[:], op=ALU.add)

            for s in range(NSTR):
                Scur = S8[s % 2]
                for kb in range(4):
                    ksub = 4 * s + kb
                    for pp_ in range(2):
                        gi = s * 8 + kb * 2 + pp_
                        gl = kb * 2 + pp_
                        if s < NSTR - 1:
                            produce_j(s + 1, 2 * gl)
                            produce_j(s + 1, 2 * gl + 1)
                        pa_zu(gi)
                        if gi in REST_G:
                            rest_q(REST_G[gi])
                        if gi in FOLD_G:
                            fold_q(FOLD_G[gi])
                        MT2 = ps_mt2.tile([P, 2 * KS], f32, tag="MT2",
                                          name="MT2")
                        for j in range(NS2):
                            stat = Scur[j][:, :, kb * P:(kb + 1) * P]
                            for h in range(2):
                                m0 = pp_ * 1024 + h * KS
                                nc.tensor.matmul(
                                    MT2[:, h * KS:(h + 1) * KS], stat,
                                    vsT[j][:, :, m0:m0 + KS],
                                    start=(j == 0), stop=(j == NS2 - 1),
                                    perf_mode=DR)
                            if j == 5 and pend:
                                flush_one()
                            if j == 11 and pend:
                                flush_one()
                        Et2 = expp.tile([P, 2 * KS], bf16, tag="E",
                                        name="Et2")
                        nc.scalar.activation(Et2[:], MT2[:], AF.Exp,
                                             bias=nshift[:, 0:1], scale=0.5)
                        for h in range(2):
                            pend.append((ksub, pp_ * 2 + h, Et2, h))

            if DEBUG:
                nc.sync.dma_start(d_dbg_u[:], uT[:])
                nc.sync.dma_start(d_dbg_rest[:], restT[:])

            while pend:
                flush_one()

            if DEBUG:
                nc.sync.dma_start(d_dbg_xeT[:], xeT[:])

            # ---------- epilogue: fT = tanh(restT + xeT[:F]/l) ----------
            # l row -> (128,512) per chunk via K=1 ones matmul (f32 moving),
            # then DVE reciprocal (full-partition, ~0.3us/chunk).
            for q in range(4):
                sl = slice(q * KS, (q + 1) * KS)
                lps = ps_sh.tile([P, KS], f32, tag="sh", name="lps")
                nc.tensor.matmul(lps[:], ones1f[F:F + 1, :], xeT[F:F + 1, sl],
                                 start=True, stop=True)
                linv = bcp.tile([P, KS], f32, tag="bcf", name="linv")
                nc.vector.reciprocal_approx_fast(linv[:], lps[:])
                xf = workp.tile([F, KS], f32, tag="fin", name="xf")
                nc.vector.tensor_tensor(xf[:], xeT[:F, sl], linv[:F, :],
                                        op=ALU.mult)
                nc.vector.tensor_tensor(xf[:], xf[:], restT[:, sl],
                                        op=ALU.add)
                nc.scalar.activation(xf[:], xf[:], AF.Tanh)
                nc.sync.dma_start(d_out[:, sl], xf[:])

    nc.compile()
    return nc


def _in_maps(x, x0, alpha, beta, w, d, w1, w2, vs, bs, node_emb, conv_w,
             conv_b):
    bfl = ml_dtypes.bfloat16
    f8 = ml_dtypes.float8_e4m3
    embT = np.ascontiguousarray(node_emb.T).astype(bfl)
    cvv = np.array([[conv_w[0], conv_b[0]]], dtype=np.float32)
    bs16 = np.ascontiguousarray(bs).astype(bfl)
    Wsb = ((w * np.clip(d, 0.0, 1.0)) @ w.T).astype(np.float32)
    maps = []
    for c in range(8):
        b, h = c // 2, c % 2
        rows = slice(h * MH, (h + 1) * MH)
        xb = x[b]
        xbT = np.ascontiguousarray(xb.T)
        e1 = (xb @ w1).astype(np.float32)
        e2 = (xb @ w2).astype(np.float32)
        maps.append({
            "xb": np.ascontiguousarray(xb).astype(bfl),
            "e2b": np.ascontiguousarray(
                np.broadcast_to(e2.astype(bfl)[None, :], (P, N))),
            "e1c": np.ascontiguousarray(e1.reshape(XT, P).T),
            "Wsb": Wsb,
            "xhT": np.ascontiguousarray(xbT[:, rows]),
            "x0T": np.ascontiguousarray(x0[b].T[:, rows]),
            "alr": np.ascontiguousarray(alpha[rows])[None, :].astype(bfl),
            "ber": np.ascontiguousarray(beta[rows])[None, :].astype(bfl),
            "conv2": cvv,
            "vs8": np.ascontiguousarray(vs[rows].T).astype(f8),
            "bs16": bs16,
            "embT": embT,
            "emb_hT": np.ascontiguousarray(node_emb[rows].T).astype(bfl),
        })
    return maps


def kernel(**inputs):
    inputs = {k: np.asarray(v) for k, v in inputs.items()}
    x = inputs["x"].astype(np.float32)
    if "nc" not in _CACHE:
        _CACHE["nc"] = build_nc()
    nc = _CACHE["nc"]
    maps = _in_maps(
        x, inputs["x0"].astype(np.float32), inputs["alpha"].astype(np.float32),
        inputs["beta"].astype(np.float32), inputs["w"].astype(np.float32),
        inputs["d"].astype(np.float32), inputs["w1"].astype(np.float32),
        inputs["w2"].astype(np.float32), inputs["vs"].astype(np.float32),
        inputs["bs"].astype(np.float32), inputs["node_emb"].astype(np.float32),
        inputs["conv_w"].astype(np.float32),
        inputs["conv_b"].astype(np.float32))
    res = run_bass_kernel_spmd(nc, maps, core_ids=list(range(8)))
    out = np.empty((B, N, F), dtype=np.float32)
    for c in range(8):
        b, h = c // 2, c % 2
        out[b, h * MH:(h + 1) * MH] = np.asarray(res.results[c]["out"]).T
    return out
